# revision 19
# baseline (speedup 1.0000x reference)
"""DeltaNet Trainium2 kernel (nn_DeltaNet_41961830482331).

Full module: qkv = x @ w_attn; per-(head,dim-group) standardization (ddof=1);
DeltaNet recurrence  S_t = S_{t-1}(0.99 I - 0.01 k k^T) + k v^T, o_t = S_t q_t;
y = o @ w_proj; out = x + y.

Sharding: 8 cores = 4 batches x 2 head-groups (6 heads each). Each core runs
the full pipeline for its (batch, head-group); host sums the two partial
y-projections per batch (w_proj is row-split across the head-group pair).

Recurrence math (chunked, chunk n=128, gamma=0.99, beta=0.01):
substituting S_t = g^t Sh_t turns the decayed update into plain DeltaNet
  Sh_t = Sh_{t-1}(I - b' k k^T) + k nu_t^T,  b' = beta/g, nu_t = g^-t v_t,
  o_t = Sh_t qh_t, qh_t = g^t q_t.
Per chunk (K rows k_t, Vh rows nu_t, Qh rows qh_t, start state Sh0):
  N   = b' stril(K K^T)
  M   = (I + N)^{-1} (b'(stril(K Vh^T) K + K Sh0^T))
  O   = tril(Qh Vh^T) K - tril(Qh K^T) M + Qh Sh0^T
  Shn = Sh0 + K^T Vh - M^T K ;  next Sh0 = g^n Shn
The triangular solve uses the exact-through-N^7 factorization
  (I + N)^{-1} ~= (I - N)(I + N^2)(I + N^4)
with (I+N^4)v computed as v + N^2(N^2 v); signs are folded so the result is
-M directly.

Engine notes: Pool (nc.gpsimd) cannot access PSUM, so all PSUM evacuation and
mask-fusing goes through ACT (nc.scalar) / DVE (nc.vector); Pool handles
SBUF-only work (squares, small stat arithmetic). Group mean sums come from an
extra tiny matmul against host-precomputed row-sums of w_attn
(sum_d qkv = x @ wAg).

Schedule: the PE instruction queue is in-order, so chunk c+1's projection
matmuls are interleaved INTO chunk c's recurrence at the points where the
recurrence waits on vector/scalar results (solve adds, evacs).  This keeps
the PE streak long (p-state ramps to full clock) and hides the stats chain.
"""

import numpy as np

B, T, C = 4, 1024, 768
NH, HS = 12, 64
HPC = NH // 2            # heads per core
GAMMA, BETA = 0.99, 0.01
BP = BETA / GAMMA        # beta'
NC_ = 128                # chunk length n
NCH = T // NC_           # chunks
GN = GAMMA ** NC_        # gamma^n
W3 = 3 * HPC * HS        # 1152
KT = C // 128            # 6 contraction tiles for qkv proj
KP = HPC * HS // 128     # 3 contraction tiles for out proj

_cache: dict = {}


def _build_program():
    import concourse.bass as bass
    import concourse.tile as tile
    from concourse import bacc, mybir

    f32 = mybir.dt.float32
    bf16 = mybir.dt.bfloat16
    Alu = mybir.AluOpType
    Act = mybir.ActivationFunctionType

    nc = bacc.Bacc()

    # ---- DRAM parameters (per-core data; SPMD: same names on all cores) ----
    xT = nc.dram_tensor("xT", [C, T], bf16, kind="ExternalInput")          # x[b].T
    wA = nc.dram_tensor("wA", [C, W3], bf16, kind="ExternalInput")
    wP = nc.dram_tensor("wP", [HPC * HS, C], bf16, kind="ExternalInput")
    xres = nc.dram_tensor("xres", [T, C], bf16, kind="ExternalInput")      # x[b] or 0
    gvec = nc.dram_tensor("gvec", [128, 2], f32, kind="ExternalInput")     # g^(p+1), g^-(p+1)
    wAg = nc.dram_tensor("wAg", [C, 18], bf16, kind="ExternalInput")       # 64-group col sums of wA
    # consts = [id | SU | SL | M3] with M3 = [IU | SU | IU] (384 wide)
    consts = nc.dram_tensor("consts", [128, 768], bf16, kind="ExternalInput")
    y = nc.dram_tensor("y", [T, C], f32, kind="ExternalOutput")

    with tile.TileContext(nc) as tc:
        with (
            tc.tile_pool(name="persist", bufs=1) as persist,
            tc.tile_pool(name="qkvp", bufs=2) as qkvp,
            tc.tile_pool(name="statp", bufs=2) as statp,
            tc.tile_pool(name="natp", bufs=2) as natp,
            tc.tile_pool(name="tp", bufs=2) as tp,
            tc.tile_pool(name="gramp", bufs=2) as gramp,
            tc.tile_pool(name="solvep", bufs=2) as solvep,
            tc.tile_pool(name="stp", bufs=2) as stp,
            tc.tile_pool(name="yp", bufs=2) as yp,
            tc.tile_pool(name="ps_a", bufs=2, space="PSUM") as ps_a,
            tc.tile_pool(name="ps_g", bufs=3, space="PSUM") as ps_g,
            tc.tile_pool(name="ps_s", bufs=3, space="PSUM") as ps_s,
        ):
            # ---- persistent operands; chunk-0 columns of xT staged first so
            # the first projection can start ~1.5us in ----
            xT_sb = persist.tile([128, KT, T], bf16)
            wA_sb = persist.tile([128, KT, W3], bf16)
            xTr = xT.rearrange("(k p) t -> p k t", p=128)
            wAr = wA.rearrange("(k p) j -> p k j", p=128)
            for k in range(KT):
                nc.sync.dma_start(out=xT_sb[:, k, 0:NC_], in_=xTr[:, k, 0:NC_])
                nc.sync.dma_start(out=wA_sb[:, k, 384:768], in_=wAr[:, k, 384:768])
            for k in range(KT):
                nc.sync.dma_start(out=xT_sb[:, k, NC_:T], in_=xTr[:, k, NC_:T])
            cs_sb = persist.tile([128, 768], bf16)
            nc.gpsimd.dma_start(out=cs_sb, in_=consts[:, :])
            gv_sb = persist.tile([128, 2], f32)
            nc.gpsimd.dma_start(out=gv_sb, in_=gvec[:, :])
            wAg_sb = persist.tile([128, KT, 18], bf16)
            nc.gpsimd.dma_start(out=wAg_sb, in_=wAg.rearrange("(k p) j -> p k j", p=128))
            wAb = wA_sb.rearrange("p k (b j) -> p k b j", b=3)
            wArb = wAr.rearrange("p k (b j) -> p k b j", b=3)
            for k in range(KT):
                nc.gpsimd.dma_start(out=wAb[:, k, 0:3:2, :], in_=wArb[:, k, 0:3:2, :])
            wP_sb = persist.tile([128, KP, C], bf16)
            nc.gpsimd.dma_start(out=wP_sb, in_=wP.rearrange("(k p) j -> p k j", p=128))
            xr_sb = persist.tile([128, NCH, C], bf16)
            nc.gpsimd.dma_start(out=xr_sb, in_=xres.rearrange("(n p) c -> p n c", p=128))

            id_sb = cs_sb[:, 0:128]
            mSU = cs_sb[:, 128:256]   # strict upper (j>i)
            mSL = cs_sb[:, 256:384]   # strict lower (i>j)
            m3 = cs_sb[:, 384:768]    # [IU | SU | IU] for [F2T | HvT | FiT]

            # O^T for the whole sequence: [ch=384, t=1024]
            outT_sb = persist.tile([128, KP, T], bf16)

            st_prev = stp.tile([128, 192], bf16)
            nc.vector.memset(st_prev, 0.0)

            GOFF = {"q": 0, "k": 6, "v": 12}
            BCOL = {"q": 0, "k": 384, "v": 768}

            def emit_s64(c):
                """PE group-sums for all 18 qkv 64-groups: x @ rowsum(wA)."""
                t0 = c * NC_
                s64ps = ps_s.tile([128, 18], f32, tag="sm")
                for k in range(KT):
                    nc.tensor.matmul(s64ps[:, :], lhsT=xT_sb[:, k, t0:t0 + 128],
                                     rhs=wAg_sb[:, k, :],
                                     start=(k == 0), stop=(k == KT - 1))
                s64a = statp.tile([128, 18], f32, tag="s64")
                nc.scalar.copy(s64a[:, :], s64ps[:, :])
                return s64a

            def emit_block_proj(c, st, bname):
                """One qkv block (384 cols): project + evacuate."""
                t0 = c * NC_
                c0 = BCOL[bname]
                qkv_sb = st["qkv_sb"]
                pp = ps_a.tile([128, 384], f32, tag="qkvps")
                for k in range(KT):
                    nc.tensor.matmul(
                        pp[:, :],
                        lhsT=xT_sb[:, k, t0:t0 + 128],
                        rhs=wA_sb[:, k, c0:c0 + 384],
                        start=(k == 0), stop=(k == KT - 1),
                    )
                nc.scalar.copy(qkv_sb[:, c0:c0 + 384], pp[:, :])

            def emit_block_stats(c, st, bname):
                """Stats + normalization for one qkv block (SBUF only).
                k-norms run on ACT; v/q-norms on DVE so they cannot delay
                ACT-side evacs of the current recurrence."""
                c0 = BCOL[bname]
                qkv_sb = st["qkv_sb"]
                qc = qkv_sb[:, c0:c0 + 384]
                s64 = st["s64a"][:, GOFF[bname]:GOFF[bname] + 6]
                sq = statp.tile([128, 384], bf16, tag=f"sq_{bname}")
                nc.gpsimd.tensor_tensor(sq, qc, qc, op=Alu.mult)
                m2 = statp.tile([128, 6], f32, tag=f"m2_{bname}")
                nc.vector.tensor_reduce(
                    m2, sq.rearrange("p (g d) -> p g d", d=64),
                    axis=mybir.AxisListType.X, op=Alu.add)
                # M2 = sum(x^2) - (sum x)^2/64 ; var_unbiased = M2/63
                t1 = statp.tile([128, 6], f32, tag=f"t1_{bname}")
                nc.gpsimd.tensor_tensor(t1, s64, s64, op=Alu.mult)
                nc.vector.scalar_tensor_tensor(
                    out=m2, in0=t1, scalar=-1.0 / 64.0, in1=m2,
                    op0=Alu.mult, op1=Alu.add)
                rstd = statp.tile([128, 6], f32, tag=f"rstd_{bname}")
                nc.scalar.activation(rstd, m2, Act.Sqrt, scale=1.0 / 63.0)
                nc.vector.reciprocal(rstd, rstd)
                if bname == "k":
                    rsb = rstd
                else:
                    gvc = gv_sb[:, 0:1] if bname == "q" else gv_sb[:, 1:2]
                    rsb = statp.tile([128, 6], f32, tag=f"rs_{bname}")
                    nc.vector.tensor_tensor(rsb, rstd,
                                            gvc.to_broadcast((128, 6)),
                                            op=Alu.mult)
                bi = statp.tile([128, 6], f32, tag=f"bias_{bname}")
                nc.vector.scalar_tensor_tensor(
                    out=bi, in0=s64, scalar=-1.0 / 64.0, in1=rsb,
                    op0=Alu.mult, op1=Alu.mult)
                st[f"rs_{bname}"] = rsb
                st[f"bi_{bname}"] = bi

            def emit_block_norms(st, bname):
                c0 = BCOL[bname]
                qkv_sb = st["qkv_sb"]
                rsb = st[f"rs_{bname}"]
                bi = st[f"bi_{bname}"]
                tiles = []
                for p in range(HPC // 2):
                    nt = natp.tile([128, 128], bf16, tag=f"{bname}np{p}")
                    for sub in range(2):
                        i = 2 * p + sub
                        sl = slice(64 * sub, 64 * sub + 64)
                        src = qkv_sb[:, c0 + 64 * i:c0 + 64 * i + 64]
                        if bname == "k":
                            nc.scalar.activation(
                                nt[:, sl], src,
                                Act.Identity, bias=bi[:, i:i + 1],
                                scale=rsb[:, i:i + 1])
                        else:
                            nc.vector.tensor_scalar(
                                out=nt[:, sl], in0=src,
                                scalar1=rsb[:, i:i + 1], scalar2=bi[:, i:i + 1],
                                op0=Alu.mult, op1=Alu.add)
                    tiles.append(nt)
                st[bname] = tiles

            def emit_block(c, st, bname):
                emit_block_proj(c, st, bname)
                emit_block_stats(c, st, bname)
                emit_block_norms(st, bname)

            def emit_qkv_start(c):
                qkv_t = qkvp.tile([128, W3], bf16, tag="qkv", name="qkv_t")
                st = {"qkv_sb": qkv_t, "s64a": emit_s64(c)}
                emit_block(c, st, "k")
                return st

            def emit_rec(c, cur, nxt):
                """recurrence for chunk c, interleaved with chunk c+1's qkv
                projection blocks (nxt is the partial state or None)."""
                nonlocal st_prev
                t0 = c * NC_
                knp_l, vnp_l, qnp_l = cur["k"], cur["v"], cur["q"]

                # --- transposes: [K^T | Q^T | V^T] per pair ---
                tsb_l = []
                for p in range(HPC // 2):
                    tps = ps_s.tile([128, 384], bf16, tag="sm")
                    nc.tensor.transpose(tps[:, 128:256], qnp_l[p][:, :], id_sb)
                    nc.tensor.transpose(tps[:, 256:384], vnp_l[p][:, :], id_sb)
                    nc.tensor.transpose(tps[:, 0:128], knp_l[p][:, :], id_sb)
                    tsb = tp.tile([128, 384], bf16, tag=f"tsb{p}")
                    nc.scalar.copy(tsb[:, :], tps[:, :])
                    tsb_l.append(tsb)

                # --- gram products; gsb = [N_up | N_low | F2T | HvT | FiT] ---
                gram_l = []
                for p in range(HPC // 2):
                    tsb = tsb_l[p]
                    for sub in range(2):
                        i = 2 * p + sub
                        po = 64 * sub
                        kt = tsb[po:po + 64, 0:128]
                        vt = tsb[po:po + 64, 256:384]
                        kq = tsb[po:po + 64, 0:256]      # [K^T | Q^T]
                        gps = ps_g.tile([128, 512], f32, tag="gram")
                        nc.tensor.matmul(gps[:, 0:256], lhsT=kt, rhs=kq,
                                         tile_position=(po, 0))
                        nc.tensor.matmul(gps[:, 256:512], lhsT=vt, rhs=kq,
                                         tile_position=(po, 0))
                        # gps = [G | F2T_raw | HvT_raw | FiT_raw]
                        gsb = gramp.tile([128, 640], bf16, tag=f"gsb{i}")
                        nc.vector.scalar_tensor_tensor(
                            out=gsb[:, 0:128], in0=gps[:, 0:128], scalar=-BP,
                            in1=mSU, op0=Alu.mult, op1=Alu.mult)
                        nc.vector.scalar_tensor_tensor(
                            out=gsb[:, 128:256], in0=gps[:, 0:128], scalar=-BP,
                            in1=mSL, op0=Alu.mult, op1=Alu.mult)
                        nc.vector.tensor_tensor(
                            gsb[:, 256:640], gps[:, 128:512], m3, op=Alu.mult)
                        gram_l.append(gsb)

                # --- N2T[i] = (N^2)^T (lhsT form for the solve) ---
                n2T_l = []
                for half in range(2):
                    n2ps = ps_s.tile([128, 384], f32, tag="sm")
                    for j in range(3):
                        i = 3 * half + j
                        nc.tensor.matmul(n2ps[:, 128 * j:128 * j + 128],
                                         lhsT=gram_l[i][:, 128:256],
                                         rhs=gram_l[i][:, 0:128])
                    n2sb = solvep.tile([128, 384], bf16, tag=f"n2T{half}")
                    nc.scalar.copy(n2sb[:, :], n2ps[:, :])
                    n2T_l.append(n2sb)

                def n2mm(dst, src):
                    for i in range(HPC):
                        nc.tensor.matmul(
                            dst[:, 64 * i:64 * i + 64],
                            lhsT=n2T_l[i // 3][:, 128 * (i % 3):128 * (i % 3) + 128],
                            rhs=src[:, 64 * i:64 * i + 64])

                # --- R_raw = HvT^T K + K Sh0^T ; r = -BP * R_raw = -R ---
                rps = ps_s.tile([128, 384], f32, tag="sm")
                for i in range(HPC):
                    p, sub = divmod(i, 2)
                    po = 64 * sub
                    nc.tensor.matmul(rps[:, 64 * i:64 * i + 64],
                                     lhsT=gram_l[i][:, 384:512],
                                     rhs=knp_l[p][:, po:po + 64],
                                     start=True, stop=False)
                    nc.tensor.matmul(rps[:, 64 * i:64 * i + 64],
                                     lhsT=tsb_l[p][po:po + 64, 0:128],
                                     rhs=st_prev[po:po + 64, 64 * p:64 * p + 64],
                                     start=False, stop=True, tile_position=(po, 0))
                r_sb = solvep.tile([128, 384], bf16, tag="rsb")
                nc.scalar.mul(r_sb[:, :], rps[:, :], -BP)

                # fill the r_sb wait with chunk c+1's group sums + k block
                if nxt is not None:
                    nxt["s64a"] = emit_s64(c + 1)
                    emit_block_proj(c + 1, nxt, "k")
                    emit_block_stats(c + 1, nxt, "k")

                # --- u1 = N R - R ---
                nrps = ps_s.tile([128, 384], f32, tag="sm")
                for i in range(HPC):
                    nc.tensor.matmul(nrps[:, 64 * i:64 * i + 64],
                                     lhsT=gram_l[i][:, 0:128],
                                     rhs=r_sb[:, 64 * i:64 * i + 64])
                u1 = solvep.tile([128, 384], bf16, tag="u1")
                nc.vector.tensor_tensor(u1[:, :], nrps[:, :], r_sb[:, :], op=Alu.add)

                if nxt is not None:
                    emit_block_proj(c + 1, nxt, "v")
                    emit_block_stats(c + 1, nxt, "v")

                # --- v1 = (I+N^2) u1 ---
                vps = ps_s.tile([128, 384], f32, tag="sm")
                n2mm(vps, u1)
                v1 = solvep.tile([128, 384], bf16, tag="v1")
                nc.vector.tensor_tensor(v1[:, :], vps[:, :], u1[:, :], op=Alu.add)

                # --- mneg = v1 + N^2 (N^2 v1) = (I+N^4) v1 = -M ---
                tps2 = ps_s.tile([128, 384], f32, tag="sm")
                n2mm(tps2, v1)
                tmp = solvep.tile([128, 384], bf16, tag="tmp")
                nc.vector.tensor_copy(tmp[:, :], tps2[:, :])

                # q projection fills the tmp-evac wait; its stats/norms are
                # deferred past the O emission so they cannot block ACT evacs
                if nxt is not None:
                    emit_block_proj(c + 1, nxt, "q")

                mps = ps_s.tile([128, 384], f32, tag="sm")
                n2mm(mps, tmp)
                mneg = solvep.tile([128, 384], bf16, tag="mneg")
                nc.vector.tensor_tensor(mneg[:, :], mps[:, :], v1[:, :], op=Alu.add)

                # --- O^T = K^T FiT + Sh0 Qh^T + Mneg^T F2T (one evac) ---
                o_ps = ps_s.tile([128, 384], f32, tag="sm")
                for p in range(HPC // 2):
                    for sub in range(2):
                        i = 2 * p + sub
                        po = 64 * sub
                        sl = slice(po, po + 64)
                        ow = o_ps[sl, 128 * p:128 * p + 128]
                        nc.tensor.matmul(ow, lhsT=knp_l[p][:, sl],
                                         rhs=gram_l[i][:, 512:640],
                                         start=True, stop=False, tile_position=(0, po))
                        nc.tensor.matmul(ow,
                                         lhsT=st_prev[sl, 64 * p:64 * p + 64],
                                         rhs=tsb_l[p][sl, 128:256],
                                         start=False, stop=False, tile_position=(po, po))
                        nc.tensor.matmul(ow,
                                         lhsT=mneg[:, 64 * i:64 * i + 64],
                                         rhs=gram_l[i][:, 256:384],
                                         start=False, stop=True, tile_position=(0, po))
                nc.scalar.copy(
                    outT_sb[:, :, t0:t0 + 128],
                    o_ps.rearrange("p (k t) -> p k t", k=KP))

                if nxt is not None:
                    emit_block_norms(nxt, "k")
                    emit_block_norms(nxt, "v")
                    emit_block_stats(c + 1, nxt, "q")
                    emit_block_norms(nxt, "q")

                # --- state: Shn^T = Sh0^T + Vh^T K + K^T Mneg ---
                sps = ps_s.tile([128, 192], f32, tag="sm")
                for i in range(HPC):
                    p, sub = divmod(i, 2)
                    po = 64 * sub
                    psl = slice(po, po + 64)
                    fsl = slice(64 * p, 64 * p + 64)
                    nc.tensor.matmul(sps[psl, fsl], lhsT=vnp_l[p][:, psl],
                                     rhs=knp_l[p][:, psl],
                                     start=True, stop=False, tile_position=(0, po))
                    nc.tensor.matmul(sps[psl, fsl], lhsT=knp_l[p][:, psl],
                                     rhs=mneg[:, 64 * i:64 * i + 64],
                                     start=False, stop=False, tile_position=(0, po))
                    nc.tensor.matmul(sps[psl, fsl], lhsT=id_sb[psl, psl],
                                     rhs=st_prev[psl, fsl],
                                     start=False, stop=True, tile_position=(po, po))
                st_new = stp.tile([128, 192], bf16)
                nc.scalar.mul(st_new[:, :], sps[:, :], GN)
                st_prev = st_new

            def emit_proj(c):
                """output projection + residual + store for chunk c."""
                t0 = c * NC_
                y_sb = yp.tile([128, C], f32, tag="ysb")
                for nblk in range(2):
                    ypp = ps_a.tile([128, 384], f32, tag="qkvps")
                    for k in range(KP):
                        nc.tensor.matmul(
                            ypp[:, :],
                            lhsT=outT_sb[:, k, t0:t0 + 128],
                            rhs=wP_sb[:, k, 384 * nblk:384 * nblk + 384],
                            start=(k == 0), stop=(k == KP - 1),
                        )
                    nc.vector.tensor_tensor(y_sb[:, 384 * nblk:384 * nblk + 384],
                                            ypp[:, :],
                                            xr_sb[:, c, 384 * nblk:384 * nblk + 384],
                                            op=Alu.add)
                nc.sync.dma_start(out=y[t0:t0 + 128, :], in_=y_sb[:, :])

            cur = emit_qkv_start(0)
            emit_block(0, cur, "v")
            emit_block(0, cur, "q")
            for c in range(NCH):
                if c + 1 < NCH:
                    qkv_n = qkvp.tile([128, W3], bf16, tag="qkv", name="qkv_n")
                    nxt = {"qkv_sb": qkv_n}
                else:
                    nxt = None
                emit_rec(c, cur, nxt)
                emit_proj(c)
                cur = nxt

    nc.finalize()
    return nc


def _host_inputs(x, w_attn, w_proj):
    """Build the 8 per-core input maps."""
    import ml_dtypes
    bf = ml_dtypes.bfloat16
    in_maps = []
    gvec = np.zeros((128, 2), np.float32)
    p = np.arange(1, 129, dtype=np.float64)
    gvec[:, 0] = GAMMA ** p
    gvec[:, 1] = GAMMA ** (-p)
    ii, jj = np.indices((128, 128))
    mSU = (jj > ii).astype(np.float32)
    mSL = (ii > jj).astype(np.float32)
    mIU = (jj >= ii).astype(np.float32)
    consts = np.concatenate([
        np.eye(128, dtype=np.float32), mSU, mSL,
        mIU, mSU, mIU,                      # M3 = [IU | SU | IU]
    ], axis=1).astype(bf)
    for core in range(8):
        b, hg = divmod(core, 2)
        h0 = hg * HPC
        cols = []
        for blk in range(3):   # q, k, v column blocks of w_attn
            cols.append(w_attn[:, blk * C + h0 * HS: blk * C + (h0 + HPC) * HS])
        wA_s = np.ascontiguousarray(np.concatenate(cols, axis=1)).astype(bf)
        wP_s = np.ascontiguousarray(w_proj[h0 * HS:(h0 + HPC) * HS]).astype(bf)
        xb = np.ascontiguousarray(x[b])                                # [1024, 768]
        xres = xb.astype(bf) if hg == 0 else np.zeros((T, C), bf)
        wAg = np.ascontiguousarray(
            wA_s.astype(np.float32).reshape(C, 18, 64).sum(axis=2)).astype(bf)
        in_maps.append({
            "xT": np.ascontiguousarray(xb.T).astype(bf),
            "wA": wA_s,
            "wAg": wAg,
            "wP": wP_s,
            "xres": xres,
            "gvec": gvec,
            "consts": consts,
        })
    return in_maps


def kernel(x, w_attn, w_proj):
    from concourse.bass_utils import run_bass_kernel_spmd

    if "nc" not in _cache:
        _cache["nc"] = _build_program()
    nc = _cache["nc"]

    in_maps = _host_inputs(np.asarray(x), np.asarray(w_attn), np.asarray(w_proj))
    res = run_bass_kernel_spmd(nc, in_maps, core_ids=list(range(8)))
    out = np.empty((B, T, C), np.float32)
    for b in range(B):
        out[b] = res.results[2 * b]["y"] + res.results[2 * b + 1]["y"]
    return out
